# revision 1
# baseline (speedup 1.0000x reference)
"""DHVT block kernel for Trainium2, 8 NeuronCores, data-parallel over batch.

Full inputs in, full outputs out. Each core processes B_LOCAL=16 images.

Per-core pipeline (all matmuls bf16 with fp32 PSUM accumulation):
  A. load x (q-aligned 128+76 blocks/img), LN1 stats, x-hat (bf16),
     xb = x + proj_b (fp32, becomes x1 in-place later)
  B. PE-transpose x-hat -> xa^T (gamma/beta fused in eviction, bf16)
  C. head-token branch (mean over tokens -> ht proj -> LN -> GELU -> +pos),
     transposed into xa^T columns 197..202 per image
  D. qkv: Q^T,K^T feature-major (weights stationary); V token-major
     (xa^T stationary) with a ones-column per head block for softmax denom
  E. attention per (img, head): S^T = K@Q^T, exp (scale fused), PV with
     ones-col -> unnormalized out^T + denominator row; reciprocal +
     DMA-broadcast; TT-mult eviction into attn^T
  F. proj (attn^T stationary) + mean-of-head-tokens column; evict as
     x1 = x + proj_b + proj_out in place of xb
  G. LN2 + transpose -> xn2^T (gamma2/beta2 fused, bf16)
  H. DAFF per 2-image chunk: conv1 (GELU+BN-fold fused eviction),
     depthwise 3x3 as 9 diagonal matmuls in PSUM, conv3 (+BN fold,
     SE sums via accum_out), squeeze-excite, final residual + DMA out
"""

import numpy as np
import ml_dtypes

import bass_rust
import concourse.bass as bass
import concourse.tile as tile
from concourse import mybir
from concourse.vector_clock import ScopedClock

FP32 = mybir.dt.float32
BF16 = mybir.dt.bfloat16
ALU = mybir.AluOpType
ACTF = mybir.ActivationFunctionType
AXIS = mybir.AxisListType

B, N, C = 128, 197, 384
H, HD = 6, 64
HID = 1536
S = 14
SCALE = HD ** -0.5
N_CORES = 8
BL = B // N_CORES          # images per core = 16
M = N + H                  # 203 tokens incl head tokens
NB = 2 * BL                # q-aligned blocks (128 + 76 rows per image)
CT = N - 1                 # conv tokens per image = 196
EPS = 1e-5


class PatchedTileContext(tile.TileContext):
    """Workaround: this walrus build accepts at most 1 sync wait on a plain
    SP instruction, but the stock tail drain carries the whole residual
    clock. Redistribute the waits one-per-nop before a bare drain."""

    def _drain_and_barrier(self, tick_clock, wait_clock):
        nc = self.nc
        collector = nc.sync.nop()
        wait_clock.add_sem_waits(
            collector.ins, ScopedClock({None: tick_clock.global_clock})
        )
        si = collector.ins.sync_info
        waits = list(si.on_wait) if si is not None else []
        if len(waits) > 1:
            si.on_wait = waits[:1]
            for w in waits[1:]:
                n = nc.sync.nop()
                n.ins.sync_info = bass_rust.SyncInfo(on_wait=[w], on_update=[])
        nc.sync.drain()
        nc.all_engine_barrier()
        assert self.sems is not None
        popped = nc._tile_sem_poison_stack.pop()
        assert popped is self._sem_poison
        nc.clear_and_free_semaphores(list(self.sems.allocated().values()))
        nc.all_engine_barrier()


def _bf(a):
    return np.ascontiguousarray(a.astype(ml_dtypes.bfloat16))


def _f32(a):
    return np.ascontiguousarray(np.asarray(a, dtype=np.float32))


def host_constants(inputs):
    """Fold BN/LN constants and lay out weights for the device program."""
    g = {k: np.asarray(v, dtype=np.float32) for k, v in inputs.items()}
    s_bn = 1.0 / np.sqrt(1.0 + EPS)   # running_var=1 eval BatchNorm scale
    s1 = g["bn1_g"] * s_bn
    s2 = g["bn2_g"] * s_bn
    s3 = g["bn3_g"] * s_bn

    consts = {}
    # qkv: Q,K stationary [k, ktile, m]; V moving [k, ktile, m]
    wqk = g["qkv_w"][:, : 2 * C]                       # [384, 768]
    consts["wqk"] = _bf(wqk.reshape(3, 128, 2 * C).transpose(1, 0, 2))
    consts["bqk_col"] = _f32(g["qkv_b"][: 2 * C].reshape(6, 128).T)  # [128, 6]
    wv = g["qkv_w"][:, 2 * C:]                          # [384, 384]
    consts["wv"] = _bf(wv.reshape(3, 128, C).transpose(1, 0, 2))
    consts["bv_rep"] = _f32(np.tile(g["qkv_b"][2 * C:][None, :], (128, 1)))

    consts["projw"] = _bf(g["proj_w"].reshape(3, 128, C).transpose(1, 0, 2))
    consts["projb_row"] = _f32(g["proj_b"][None, :])    # [1, 384]

    # LN1/LN2 gamma/beta as per-partition columns [128, 3]
    consts["g1_col"] = _f32(g["ln1_g"].reshape(3, 128).T)
    consts["b1_col"] = _f32(g["ln1_b"].reshape(3, 128).T)
    consts["g2_col"] = _f32(g["ln2_g"].reshape(3, 128).T)
    consts["b2_col"] = _f32(g["ln2_b"].reshape(3, 128).T)

    # head-token branch
    consts["wht"] = _bf(g["ht_w"])                      # [64, 384]
    consts["htb_rep"] = _f32(np.tile(g["ht_b"][None, :], (16, 1)))  # [16,384]
    consts["htng_rep"] = _f32(np.tile(np.tile(g["htn_g"], H)[None, :], (96, 1)))
    consts["htnb_rep"] = _f32(np.tile(np.tile(g["htn_b"], H)[None, :], (96, 1)))
    # pos_embed rows ordered (h, img)
    pos = np.repeat(g["pos_embed"][0], BL, axis=0)      # [96, 384]
    consts["pos_rep"] = _f32(pos)

    # DAFF conv1 (+bn1 fold)
    w1 = g["c1_w"] * s1[None, :]                        # [384, 1536]
    b1 = g["c1_b"] * s1 + g["bn1_b"]
    consts["w1"] = _bf(w1.reshape(3, 128, HID).transpose(1, 0, 2))
    consts["c1b_col"] = _f32(b1.reshape(12, 128).T)     # [128, 12]

    # depthwise (+bn2 fold) as diagonal stationaries [128, 12, 9, 128]
    dw = g["c2_w"][:, 0] * s2[:, None, None]            # [1536, 3, 3]
    b2 = g["c2_b"] * s2 + g["bn2_b"]
    diag = np.zeros((128, 12, 9, 128), dtype=np.float32)
    idx = np.arange(128)
    for pt in range(12):
        for t in range(9):
            diag[idx, pt, t, idx] = dw[pt * 128 + idx, t // 3, t % 3]
    consts["dwdiag"] = _bf(diag)
    consts["c2b_col"] = _f32(b2.reshape(12, 128).T)

    # conv3 (+bn3 fold)
    w3 = g["c3_w"] * s3[None, :]                        # [1536, 384]
    b3 = g["c3_b"] * s3 + g["bn3_b"]
    consts["w3"] = _bf(w3.reshape(12, 128, C).transpose(1, 0, 2))
    consts["c3b_col"] = _f32(b3.reshape(3, 128).T)

    # squeeze-excite (fold the 1/196 spatial mean into cmp_w)
    consts["cmpw"] = _bf((g["cmp_w"] / CT).reshape(3, 128, 96).transpose(1, 0, 2))
    consts["cmpb_rep"] = _f32(np.tile(g["cmp_b"][None, :], (16, 1)))
    consts["excw"] = _bf(g["exc_w"])                    # [96, 384]
    consts["excb_rep"] = _f32(np.tile(g["exc_b"][None, :], (16, 1)))

    consts["g2_rep"] = _f32(np.tile(g["ln2_g"][None, :], (16, 1)))
    consts["b2_rep"] = _f32(np.tile(g["ln2_b"][None, :], (16, 1)))

    consts["ones_row"] = _f32(np.ones((1, 128)))
    consts["ident_bf"] = _bf(np.eye(128, dtype=np.float32))
    consts["ident_f32"] = _f32(np.eye(128, dtype=np.float32))
    consts["eps_col"] = _f32(np.full((128, 1), EPS))
    return consts


CONST_SPECS = {
    "wqk": ([128, 3, 768], BF16), "bqk_col": ([128, 6], FP32),
    "wv": ([128, 3, 384], BF16), "bv_rep": ([128, 384], FP32),
    "projw": ([128, 3, 384], BF16), "projb_row": ([1, 384], FP32),
    "g1_col": ([128, 3], FP32), "b1_col": ([128, 3], FP32),
    "g2_col": ([128, 3], FP32), "b2_col": ([128, 3], FP32),
    "wht": ([64, 384], BF16), "htb_rep": ([16, 384], FP32),
    "htng_rep": ([96, 384], FP32), "htnb_rep": ([96, 384], FP32),
    "pos_rep": ([96, 384], FP32),
    "w1": ([128, 3, 1536], BF16), "c1b_col": ([128, 12], FP32),
    "dwdiag": ([128, 12, 9, 128], BF16), "c2b_col": ([128, 12], FP32),
    "w3": ([128, 12, 384], BF16), "c3b_col": ([128, 3], FP32),
    "cmpw": ([128, 3, 96], BF16), "cmpb_rep": ([16, 96], FP32),
    "excw": ([96, 384], BF16), "excb_rep": ([16, 384], FP32),
    "g2_rep": ([16, 384], FP32), "b2_rep": ([16, 384], FP32),
    "ones_row": ([1, 128], FP32),
    "ident_bf": ([128, 128], BF16), "ident_f32": ([128, 128], FP32),
    "eps_col": ([128, 1], FP32),
}


def _split_excess_waits(nc, max_waits=1):
    """This walrus build rejects >2 sync waits on one instruction; move the
    excess onto same-engine nops inserted immediately before it."""
    for f in nc.m.functions:
        for bb in f.blocks:
            out = []
            changed = False
            for inst in bb.instructions:
                si = inst.sync_info
                waits = list(si.on_wait) if si is not None else []
                if len(waits) > max_waits:
                    head, rest = waits[:-max_waits], waits[-max_waits:]
                    for i in range(0, len(head), max_waits):
                        nop = mybir.InstNoOp(
                            name=f"{inst.name}-wsplit{i}", ins=[], outs=[])
                        nop.engine = inst.engine
                        nop.sync_info = bass_rust.SyncInfo(
                            on_wait=head[i: i + max_waits], on_update=[])
                        out.append(nop)
                    si.on_wait = rest
                    changed = True
                out.append(inst)
            if changed:
                bb.instructions = out


def build_program(split_waits=True, loop_repeats=None):
    nc = bass.Bass()
    x_d = nc.declare_dram_parameter("x", [BL * N, C], FP32, isOutput=False)
    out_d = nc.declare_dram_parameter("out", [BL, N, C], FP32, isOutput=True)
    cd = {
        name: nc.declare_dram_parameter(name, shape, dt, isOutput=False)
        for name, (shape, dt) in CONST_SPECS.items()
    }

    with PatchedTileContext(nc) as tc:
        if loop_repeats is None:
            _emit(nc, tc, x_d, out_d, cd)
        else:
            # timing mode: prime out <- x, then iterate out -> out so each
            # iteration is serially dependent on the previous
            out_flat = out_d[:].rearrange("b n c -> (b n) c")
            nc.sync.dma_start(out=out_flat, in_=x_d[:])
            with tc.For_i(0, loop_repeats, 1):
                _emit(nc, tc, out_flat, out_d, cd)
    if split_waits:
        _split_excess_waits(nc)
    return nc


def _emit(nc, tc, x_d, out_d, cd):
    # x_d may be a DRAM handle or an AP view of out_d (timing loop mode)
    from contextlib import ExitStack

    ctx = ExitStack()
    with ctx:
        # ---------------- persistent pools & constants -------------------
        consts = ctx.enter_context(tc.tile_pool(name="consts", bufs=1))
        SMALL_CONSTS = [
            "bqk_col", "bv_rep", "projb_row", "g1_col", "b1_col", "g2_col",
            "b2_col", "wht", "htb_rep", "htng_rep", "htnb_rep", "pos_rep",
            "c1b_col", "c2b_col", "c3b_col", "cmpw", "cmpb_rep", "excw",
            "excb_rep", "g2_rep", "b2_rep", "ident_bf", "ident_f32", "eps_col",
            "ones_row",
        ]
        sb = {}
        for name in SMALL_CONSTS:
            shape, dt = CONST_SPECS[name]
            sb[name] = consts.tile(shape, dt, tag=name, name="c_" + name)
            nc.sync.dma_start(out=sb[name][:], in_=cd[name][:])

        big = ctx.enter_context(tc.tile_pool(name="big", bufs=1))
        # x1: q-shifted blocks. blk(img,0) rows0..127 = tokens n 1..128;
        # blk(img,1) rows 0..67 = n 129..196, rows 68..74 scratch.
        x1q = big.tile([128, NB, C], FP32)
        xn2T = big.tile([128, 3, BL * CT], BF16)

        stats = ctx.enter_context(tc.tile_pool(name="stats", bufs=4))
        work = ctx.enter_context(tc.tile_pool(name="work", bufs=1))

        def ln_stats(x_ap, p):
            """(rstd, negmr) [p,1] fp32 for the rows of x_ap."""
            st = stats.tile([128, 6], FP32, tag="st")
            nc.vector.bn_stats(out=st[:p], in_=x_ap)
            mv = stats.tile([128, 2], FP32, tag="mv")
            nc.vector.bn_aggr(out=mv[:p], in_=st[:p])
            rstd = stats.tile([128, 1], FP32, tag="rstd")
            nc.scalar.activation(rstd[:p], mv[:p, 1:2], ACTF.Sqrt,
                                 bias=sb["eps_col"][:p], scale=1.0)
            nc.vector.reciprocal(out=rstd[:p], in_=rstd[:p])
            negmr = stats.tile([128, 1], FP32, tag="negmr")
            nc.vector.scalar_tensor_tensor(
                out=negmr[:p], in0=mv[:p, 0:1], scalar=-1.0, in1=rstd[:p],
                op0=ALU.mult, op1=ALU.mult)
            return rstd, negmr

        # proj_b broadcast to all partitions, used for xb and cls
        pb128 = work.tile([128, C], FP32, tag="pb128")
        pbr = cd["projb_row"][:]
        nc.gpsimd.dma_start(
            out=pb128[:],
            in_=bass.AP(tensor=pbr.tensor, offset=pbr.offset,
                        ap=[[0, 128]] + list(pbr.ap[1:])))
        # cls x rows (n=0 of each image)
        xcls = work.tile([BL, C], FP32, tag="xcls")
        nc.sync.dma_start(out=xcls[:], in_=x_d[0::N, :][0:BL, :])

        with ExitStack() as attn_ctx:
            wconsts = attn_ctx.enter_context(tc.tile_pool(name="wconsts", bufs=1))
            for name in ["wqk", "wv", "projw"]:
                shape, dt = CONST_SPECS[name]
                sb[name] = wconsts.tile(shape, dt, tag=name, name="c_" + name)
                nc.sync.dma_start(out=sb[name][:], in_=cd[name][:])

            abig = attn_ctx.enter_context(tc.tile_pool(name="abig", bufs=1))
            xaT = abig.tile([128, 3, BL * M], BF16)
            attnT = abig.tile([128, 3, BL * (M + 1)], BF16)
            qv_pool = attn_ctx.enter_context(tc.tile_pool(name="qv", bufs=2))

            xhat_pool = attn_ctx.enter_context(tc.tile_pool(name="xhat", bufs=3))
            abc_ctx = ExitStack()
            psum_t = abc_ctx.enter_context(
                tc.tile_pool(name="psum_t", bufs=4, space="PSUM"))
            psum_c1 = abc_ctx.enter_context(
                tc.tile_pool(name="psum_c1", bufs=2, space="PSUM"))

            # ========== Phase A+B: LN1, x-hat, transpose, xb =============
            for img in range(BL):
                for half in range(2):
                    p = 128 if half == 0 else N - 128          # 128 / 69
                    row0 = img * N + half * 128
                    xt = work.tile([128, C], FP32, tag="xt", bufs=4)
                    nc.sync.dma_start(out=xt[:p], in_=x_d[row0: row0 + p, :])
                    rstd, negmr = ln_stats(xt[:p], p)
                    xh = xhat_pool.tile([128, C], BF16, tag="xhat")
                    nc.vector.tensor_scalar(
                        out=xh[:p], in0=xt[:p], scalar1=rstd[:p],
                        scalar2=negmr[:p], op0=ALU.mult, op1=ALU.add)
                    col0 = img * M + half * 128
                    for k in range(3):
                        pt = psum_t.tile([128, 128], BF16, tag="tp")
                        nc.tensor.transpose(pt[:, :p],
                                            xh[:p, k * 128:(k + 1) * 128],
                                            sb["ident_bf"][:p, :p])
                        nc.scalar.activation(
                            xaT[:, k, col0: col0 + p], pt[:, :p], ACTF.Identity,
                            bias=sb["b1_col"][:, k: k + 1],
                            scale=sb["g1_col"][:, k: k + 1])
            # xb blocks (q-shifted by one row)
            for img in range(BL):
                for half in range(2):
                    p = 128 if half == 0 else 68
                    row0 = img * N + 1 + half * 128
                    blk = 2 * img + half
                    xq = work.tile([128, C], FP32, tag="xq", bufs=4)
                    nc.sync.dma_start(out=xq[:p], in_=x_d[row0: row0 + p, :])
                    if half == 1:
                        # zero the q=197..203 scratch rows (32-aligned start)
                        nc.vector.memset(x1q[64:76, blk, :], 0.0)
                    nc.vector.tensor_tensor(out=x1q[:p, blk, :], in0=xq[:p],
                                            in1=pb128[:p], op=ALU.add)

            # ================= Phase C: head-token branch ================
            xhm = work.tile([128, 3, BL], FP32, tag="xhm")
            for k in range(3):
                for img in range(BL):
                    nc.vector.tensor_reduce(
                        out=xhm[:, k, img: img + 1],
                        in_=xaT[:, k, img * M: img * M + N],
                        axis=AXIS.X, op=ALU.add)
            xhm64 = work.tile([64, H, BL], BF16, tag="xhm64")
            for k in range(3):
                nc.scalar.activation(xhm64[:, 2 * k, :], xhm[0:64, k, :],
                                     ACTF.Copy, scale=1.0 / N)
                nc.scalar.activation(xhm64[:, 2 * k + 1, :], xhm[64:128, k, :],
                                     ACTF.Copy, scale=1.0 / N)
            xhp = work.tile([96, C], FP32, tag="xhp")
            for h in range(H):
                php = psum_c1.tile([16, C], FP32, tag="php")
                nc.tensor.matmul(php[:], xhm64[:, h, :], sb["wht"][:],
                                 start=True, stop=True)
                hstg = work.tile([16, C], FP32, tag="hstg", bufs=2)
                nc.vector.tensor_tensor(out=hstg[:], in0=php[:],
                                        in1=sb["htb_rep"][:], op=ALU.add)
                nc.sync.dma_start(out=xhp[h * 16:(h + 1) * 16, :], in_=hstg[:])
            xhn = work.tile([96, C], FP32, tag="xhn")
            for gi in range(H):
                seg = xhp[:, gi * 64:(gi + 1) * 64]
                rstd, negmr = ln_stats(seg, 96)
                nc.scalar.activation(xhn[:, gi * 64:(gi + 1) * 64], seg,
                                     ACTF.Identity, bias=negmr[:96],
                                     scale=rstd[:96])
            nc.vector.tensor_tensor(out=xhn[:], in0=xhn[:],
                                    in1=sb["htng_rep"][:], op=ALU.mult)
            nc.vector.tensor_tensor(out=xhn[:], in0=xhn[:],
                                    in1=sb["htnb_rep"][:], op=ALU.add)
            xhg = work.tile([96, C], FP32, tag="xhg")
            nc.scalar.activation(xhg[:], xhn[:], ACTF.Gelu)
            xhf = work.tile([96, C], BF16, tag="xhf")
            nc.vector.tensor_tensor(out=xhf[:], in0=xhg[:],
                                    in1=sb["pos_rep"][:], op=ALU.add)
            for k in range(3):
                pt = psum_t.tile([128, 128], BF16, tag="tp")
                nc.tensor.transpose(pt[:, :96], xhf[:, k * 128:(k + 1) * 128],
                                    sb["ident_bf"][:96, :96])
                dst = xaT[:, k, :].rearrange("p (i c) -> p i c", c=M)[:, :, N:M]
                dstv = dst.rearrange("p i h -> p h i")
                src = pt[:, :96].rearrange("p (h i) -> p h i", h=H)
                nc.scalar.copy(dstv, src)

            abc_ctx.close()
            # ========== Phase D+E: qkv + attention per image pair ========
            de_ctx = ExitStack()
            psum_qk = de_ctx.enter_context(
                tc.tile_pool(name="psum_qk", bufs=2, space="PSUM"))
            psum_pv = de_ctx.enter_context(
                tc.tile_pool(name="psum_pv", bufs=1, space="PSUM"))
            psum_s = de_ctx.enter_context(
                tc.tile_pool(name="psum_s", bufs=2, space="PSUM"))
            psum_pa = de_ctx.enter_context(
                tc.tile_pool(name="psum_pa", bufs=3, space="PSUM"))
            attn_pool = attn_ctx.enter_context(
                tc.tile_pool(name="attn", bufs=4))
            den_pool = attn_ctx.enter_context(
                tc.tile_pool(name="den", bufs=4))
            for pair in range(BL // 2):
                qk2 = qv_pool.tile([128, 6, 2 * M], BF16, tag="qk2")
                for mt in range(6):
                    pq = psum_qk.tile([128, 2 * M], FP32, tag="pq")
                    for k in range(3):
                        nc.tensor.matmul(
                            pq[:], sb["wqk"][:, k, mt * 128:(mt + 1) * 128],
                            xaT[:, k, pair * 2 * M:(pair + 1) * 2 * M],
                            start=(k == 0), stop=(k == 2))
                    nc.scalar.activation(
                        qk2[:, mt, :], pq[:],
                        ACTF.Identity, bias=sb["bqk_col"][:, mt: mt + 1])
                v2 = qv_pool.tile([128, 4, H * 65], BF16, tag="v2")
                for ih in range(2):
                    img = 2 * pair + ih
                    for half in range(2):
                        p = 128 if half == 0 else M - 128      # 128 / 75
                        col0 = img * M + half * 128
                        pv = psum_pv.tile([128, C], FP32, tag="pv")
                        for k in range(3):
                            nc.tensor.matmul(pv[:p], xaT[:, k, col0: col0 + p],
                                             sb["wv"][:, k, :],
                                             start=(k == 0), stop=(k == 2))
                        vv = v2[:, 2 * ih + half, :].rearrange(
                            "p (h w) -> p h w", h=H)
                        nc.vector.tensor_tensor(
                            out=vv[:p, :, 0:64],
                            in0=pv[:p].rearrange("p (h d) -> p h d", h=H),
                            in1=sb["bv_rep"][:p].rearrange(
                                "p (h d) -> p h d", h=H),
                            op=ALU.add)
                        nc.vector.memset(vv[:p, :, 64:65], 1.0)
                for ih in range(2):
                    img = 2 * pair + ih
                    dc = img * (M + 1)
                    for hp in range(3):        # head pairs (2hp, 2hp+1)
                        pes = []
                        for kh in range(2):
                            pk = 128 if kh == 0 else M - 128
                            c0 = ih * M + kh * 128
                            pej = []
                            for j in range(2):
                                ps = psum_s.tile([128, M], FP32, tag="ps")
                                nc.tensor.matmul(
                                    ps[:pk],
                                    qk2[j * 64:(j + 1) * 64, 3 + hp,
                                        c0: c0 + pk],
                                    qk2[j * 64:(j + 1) * 64, hp,
                                        ih * M:(ih + 1) * M],
                                    start=True, stop=True)
                                pe = attn_pool.tile([128, M], BF16,
                                                    tag="pexp")
                                nc.scalar.activation(pe[:pk], ps[:pk],
                                                     ACTF.Exp, scale=SCALE)
                                pej.append(pe)
                            pes.append((pej, pk))
                        for j in range(2):
                            h = 2 * hp + j
                            pa = psum_pa.tile([65, M], FP32, tag="pa",
                                              name="pa")
                            for kh in range(2):
                                pej, pk = pes[kh]
                                nc.tensor.matmul(
                                    pa[:],
                                    v2[:pk, 2 * ih + kh, h * 65:(h + 1) * 65],
                                    pej[j][:pk],
                                    start=(kh == 0), stop=(kh == 1))
                            den = den_pool.tile([1, M], FP32, tag="den")
                            nc.vector.reciprocal(out=den[:], in_=pa[64:65, :])
                            # broadcast 1/den to 64 rows via a K=1 matmul,
                            # then stage to SBUF (TT allows one PSUM input)
                            denp = psum_pa.tile([64, M], FP32, tag="pa",
                                                name="denp")
                            nc.tensor.matmul(denp[:], sb["ones_row"][:1, 0:64],
                                             den[:], start=True, stop=True)
                            densb = den_pool.tile([64, M], BF16, tag="densb")
                            nc.scalar.copy(densb[:], denp[:])
                            if j == 0:
                                nc.vector.tensor_tensor(
                                    out=attnT[0:64, hp, dc: dc + M],
                                    in0=pa[0:64, :], in1=densb[:], op=ALU.mult)
                            else:
                                stg = attn_pool.tile([64, M], BF16, tag="stg")
                                nc.vector.tensor_tensor(out=stg[:],
                                                        in0=pa[0:64, :],
                                                        in1=densb[:],
                                                        op=ALU.mult)
                                nc.sync.dma_start(
                                    out=attnT[64:128, hp, dc: dc + M],
                                    in_=stg[:])
            # mean over the 6 head-token columns, per image
            for k in range(3):
                for img in range(BL):
                    c0 = img * (M + 1)
                    red = den_pool.tile([128, 1], FP32, tag="red")
                    nc.vector.tensor_reduce(out=red[:],
                                            in_=attnT[:, k, c0 + N: c0 + M],
                                            axis=AXIS.X, op=ALU.add)
                    nc.scalar.activation(attnT[:, k, c0 + M: c0 + M + 1],
                                         red[:], ACTF.Copy, scale=1.0 / H)

            de_ctx.close()
            # ====== Phase F: proj, x1 = x + proj_b + proj_out ============
            psum_f = attn_ctx.enter_context(
                tc.tile_pool(name="psum_f", bufs=2, space="PSUM"))
            for img in range(BL):
                for half in range(2):
                    p = 128 if half == 0 else 75
                    blk = 2 * img + half
                    c0 = img * (M + 1) + 1 + half * 128
                    pp = psum_f.tile([128, C], FP32, tag="pp")
                    for k in range(3):
                        nc.tensor.matmul(pp[:p], attnT[:, k, c0: c0 + p],
                                         sb["projw"][:, k, :],
                                         start=(k == 0), stop=(k == 2))
                    nc.vector.tensor_tensor(out=x1q[:p, blk, :], in0=pp[:p],
                                            in1=x1q[:p, blk, :], op=ALU.add)
            # cls: strided gather of q=0 and mean columns across images
            atv = attnT[:, :, :].rearrange("p k (i q) -> p k i q", q=M + 1)
            pcls = psum_f.tile([BL, C], FP32, tag="pcls", bufs=1)
            for k in range(3):
                nc.tensor.matmul(pcls[:], atv[:, k, :, 0], sb["projw"][:, k, :],
                                 start=(k == 0), stop=(k == 2))
            clsx = work.tile([BL, C], FP32, tag="clsx")
            nc.vector.tensor_tensor(out=clsx[:], in0=pcls[:], in1=xcls[:],
                                    op=ALU.add)
            pmean = psum_f.tile([BL, C], FP32, tag="pcls", name="pmean", bufs=1)
            for k in range(3):
                nc.tensor.matmul(pmean[:], atv[:, k, :, M], sb["projw"][:, k, :],
                                 start=(k == 0), stop=(k == 2))
            nc.vector.tensor_tensor(out=clsx[:], in0=pmean[:], in1=clsx[:],
                                    op=ALU.add)
            # both the q=0 row and the mean-of-head-tokens row carry proj_b
            nc.vector.tensor_tensor(out=clsx[:], in0=pb128[:BL], in1=clsx[:],
                                    op=ALU.add)
            nc.vector.tensor_tensor(out=clsx[:], in0=pb128[:BL], in1=clsx[:],
                                    op=ALU.add)

            # ================= Phase G: LN2 + transpose ==================
            psum_tg = attn_ctx.enter_context(
                tc.tile_pool(name="psum_tg", bufs=4, space="PSUM"))
            for img in range(BL):
                for half in range(2):
                    p = 128 if half == 0 else 68
                    blk = 2 * img + half
                    seg = x1q[:p, blk, :]
                    rstd, negmr = ln_stats(seg, p)
                    xh2 = xhat_pool.tile([128, C], BF16, tag="xh2")
                    nc.vector.tensor_scalar(
                        out=xh2[:p], in0=seg, scalar1=rstd[:p],
                        scalar2=negmr[:p], op0=ALU.mult, op1=ALU.add)
                    col0 = img * CT + half * 128
                    for k in range(3):
                        pt = psum_tg.tile([128, 128], BF16, tag="tpg")
                        nc.tensor.transpose(pt[:, :p],
                                            xh2[:p, k * 128:(k + 1) * 128],
                                            sb["ident_bf"][:p, :p])
                        nc.scalar.activation(
                            xn2T[:, k, col0: col0 + p], pt[:, :p],
                            ACTF.Identity, bias=sb["b2_col"][:, k: k + 1],
                            scale=sb["g2_col"][:, k: k + 1])

        # ================= Phase H: DAFF =================================
        with ExitStack() as daff_ctx:
            dconsts = daff_ctx.enter_context(tc.tile_pool(name="dconsts", bufs=1))
            for name in ["w1", "dwdiag", "w3"]:
                shape, dt = CONST_SPECS[name]
                sb[name] = dconsts.tile(shape, dt, tag=name, name="c_" + name)
                nc.sync.dma_start(out=sb[name][:], in_=cd[name][:])
            h_pool = daff_ctx.enter_context(tc.tile_pool(name="hbuf", bufs=2))
            y_pool = daff_ctx.enter_context(tc.tile_pool(name="ybuf", bufs=2))
            psum_c = daff_ctx.enter_context(
                tc.tile_pool(name="psum_c", bufs=2, space="PSUM"))
            psum_d = daff_ctx.enter_context(
                tc.tile_pool(name="psum_d", bufs=2, space="PSUM"))
            psum_y = daff_ctx.enter_context(
                tc.tile_pool(name="psum_y", bufs=2, space="PSUM"))
            psum_t2 = daff_ctx.enter_context(
                tc.tile_pool(name="psum_t2", bufs=2, space="PSUM"))

            sesum = work.tile([128, 3, BL], FP32, tag="sesum")
            CW = 2 * CT                                  # 392 cols per chunk
            SP = S + 2                                   # padded 16x16 planes
            # h in zero-padded planes so every depthwise tap reads a full
            # in-bounds window and writes the full PSUM region
            hpads = []
            for i in range(2):
                hp = h_pool.tile([128, 12, 2, SP * SP], BF16,
                                 name=f"hpad{i}", tag=f"hpad{i}", bufs=1)
                nc.vector.memset(hp[:], 0.0)
                hpads.append(hp)
            taps = [(0, 0)] + [(dy, dx) for dy in (-1, 0, 1)
                               for dx in (-1, 0, 1) if (dy, dx) != (0, 0)]
            for chunk in range(BL // 2):
                hp = hpads[chunk % 2]
                hpv = hp.rearrange("p k i (y x) -> p k i y x", y=SP, x=SP)
                for mt in range(12):
                    ph = psum_c.tile([128, CW], FP32, tag="ph")
                    for k in range(3):
                        nc.tensor.matmul(
                            ph[:], sb["w1"][:, k, mt * 128:(mt + 1) * 128],
                            xn2T[:, k, chunk * CW:(chunk + 1) * CW],
                            start=(k == 0), stop=(k == 2))
                    nc.scalar.activation(
                        hpv[:, mt, :, 1: S + 1, 1: S + 1],
                        ph[:].rearrange("p (i y x) -> p i y x", i=2, y=S, x=S),
                        ACTF.Gelu, bias=sb["c1b_col"][:, mt: mt + 1])
                h2b = h_pool.tile([128, 12, CW], BF16, tag="h2b")
                h2v = h2b.rearrange("p k (i y x) -> p k i y x", i=2, y=S, x=S)
                for mt in range(12):
                    pd = psum_d.tile([128, CW], FP32, tag="pd")
                    for ti, (dy, dx) in enumerate(taps):
                        t = (dy + 1) * 3 + (dx + 1)
                        nc.tensor.matmul(
                            pd[:], sb["dwdiag"][:, mt, t, :],
                            hpv[:, mt, :, 1 + dy: 1 + dy + S,
                                1 + dx: 1 + dx + S],
                            start=(ti == 0), stop=(ti == 8),
                            skip_group_check=True)
                    nc.scalar.activation(h2b[:, mt, :], pd[:], ACTF.Gelu,
                                         bias=sb["c2b_col"][:, mt: mt + 1])
                    nc.vector.tensor_tensor(
                        out=h2v[:, mt], in0=h2v[:, mt],
                        in1=hpv[:, mt, :, 1: S + 1, 1: S + 1], op=ALU.add)
                yt = y_pool.tile([128, 3, CW], BF16, tag="yt", bufs=1)
                for mt3 in range(3):
                    py = psum_y.tile([128, CW], FP32, tag="py")
                    for k in range(12):
                        nc.tensor.matmul(
                            py[:], sb["w3"][:, k, mt3 * 128:(mt3 + 1) * 128],
                            h2b[:, k, :], start=(k == 0), stop=(k == 11))
                    for ih in range(2):
                        img = 2 * chunk + ih
                        nc.scalar.activation(
                            yt[:, mt3, ih * CT:(ih + 1) * CT],
                            py[:, ih * CT:(ih + 1) * CT], ACTF.Identity,
                            bias=sb["c3b_col"][:, mt3: mt3 + 1],
                            accum_out=sesum[:, mt3, img: img + 1])
                for ih in range(2):
                    img = 2 * chunk + ih
                    for half in range(2):
                        p = 128 if half == 0 else 68
                        blk = 2 * img + half
                        s0 = half * 128
                        ot = y_pool.tile([128, C], FP32, tag="ot", bufs=1)
                        for k in range(3):
                            pt = psum_t2.tile([128, 128], BF16, tag="tpy")
                            nc.tensor.transpose(
                                pt[:p, :],
                                yt[:, k, ih * CT + s0: ih * CT + s0 + p],
                                sb["ident_bf"][:, :])
                            nc.vector.tensor_tensor(
                                out=ot[:p, k * 128:(k + 1) * 128],
                                in0=pt[:p, :],
                                in1=x1q[:p, blk, k * 128:(k + 1) * 128],
                                op=ALU.add)
                        nc.sync.dma_start(
                            out=out_d[img, 1 + s0: 1 + s0 + p, :],
                            in_=ot[:p])

            # ============== squeeze-excite + cls =========================
            sebf = work.tile([128, 3, BL], BF16, tag="sebf")
            nc.vector.tensor_copy(sebf[:], sesum[:])
            pse = psum_y.tile([16, 96], FP32, tag="py", name="pse")
            for k in range(3):
                nc.tensor.matmul(pse[:], sebf[:, k, :], sb["cmpw"][:, k, :],
                                 start=(k == 0), stop=(k == 2))
            se1 = work.tile([16, 96], FP32, tag="se1")
            nc.vector.tensor_tensor(out=se1[:], in0=pse[:],
                                    in1=sb["cmpb_rep"][:], op=ALU.add)
            se1g = work.tile([16, 96], BF16, tag="se1g")
            nc.scalar.activation(se1g[:], se1[:], ACTF.Gelu)
            pt = psum_t2.tile([128, 128], BF16, tag="tpy", name="tps")
            nc.tensor.transpose(pt[:96, :16], se1g[:, :],
                                sb["ident_bf"][:16, :16])
            se1T = work.tile([96, 16], BF16, tag="se1T")
            nc.scalar.copy(se1T[:], pt[:96, :16])
            pw = psum_y.tile([16, C], FP32, tag="py", name="pw")
            nc.tensor.matmul(pw[:], se1T[:], sb["excw"][:],
                             start=True, stop=True)
            wtok = work.tile([16, C], FP32, tag="wtok")
            nc.vector.tensor_tensor(out=wtok[:], in0=pw[:],
                                    in1=sb["excb_rep"][:], op=ALU.add)
            rstd, negmr = ln_stats(clsx[:], BL)
            cls2 = work.tile([16, C], FP32, tag="cls2")
            nc.scalar.activation(cls2[:], clsx[:], ACTF.Identity,
                                 bias=negmr[:BL], scale=rstd[:BL])
            nc.vector.tensor_tensor(out=cls2[:], in0=cls2[:],
                                    in1=sb["g2_rep"][:], op=ALU.mult)
            nc.vector.tensor_tensor(out=cls2[:], in0=cls2[:],
                                    in1=sb["b2_rep"][:], op=ALU.add)
            nc.vector.tensor_tensor(out=cls2[:], in0=cls2[:], in1=wtok[:],
                                    op=ALU.mult)
            ocls = work.tile([16, C], FP32, tag="ocls")
            nc.vector.tensor_tensor(out=ocls[:], in0=cls2[:], in1=clsx[:],
                                    op=ALU.add)
            nc.sync.dma_start(out=out_d[:, 0, :], in_=ocls[:])


_PROGRAM_CACHE = {}


def _get_program():
    if "nc" not in _PROGRAM_CACHE:
        _PROGRAM_CACHE["nc"] = build_program()
    return _PROGRAM_CACHE["nc"]


def kernel(**inputs):
    from concourse.bass_utils import run_bass_kernel_spmd

    consts = host_constants(inputs)
    x = np.asarray(inputs["x"], dtype=np.float32)
    nc = _get_program()
    in_maps = []
    for c in range(N_CORES):
        shard = np.ascontiguousarray(
            x[c * BL:(c + 1) * BL].reshape(BL * N, C))
        in_maps.append(dict(consts, x=shard))
    res = run_bass_kernel_spmd(nc, in_maps, list(range(N_CORES)))
    out = np.concatenate([res.results[c]["out"] for c in range(N_CORES)], axis=0)
    return out.astype(np.float32)

